# revision 42
# baseline (speedup 1.0000x reference)
"""HGNNConv Trainium2 kernel (8 NeuronCores) — single-NEFF, gather-one-hot.

Computes  Y = relu( D_n^{-1/2} H D_e^{-1} H^T D_n^{-1/2} (X W^T + b) )
for a hypergraph given by incidence pairs (node_idx[i], edge_idx[i]).

Strategy (ONE NEFF, 8 cores):
  Table build (device): tab[(c-1)*128 + j, :] = rsqrt(c) * e_j  (f16), for
    degree classes c in 1..NCLS, plus one zero row.  Built from an identity
    staged as integer data; rsqrt computed on device.
  Phase A (edges sharded): per incidence gather the raw x row AND the
    scaled-basis one-hot row tab[(deg(n)-1)*128 + eslot].  Per tile:
    psum_p[e,:] += oh^T @ x  and  psum_q[e] += oh^T @ 1  (out free = 1).
    Per 128-edge block: transpose p, then ef = inv_de * (p W^T + q b),
    written f16 to a DRAM bounce.
  AllGather (device) concatenates the 8 per-core ef slices.
  Phase B (nodes sharded): gather ef rows, build identity one-hots on the
    (otherwise idle) DVE per tile, accumulate per node block,
    y = relu(rsqrt(d_n) * sum).
  All row gathers are issued on uint32-reinterpreted tables (256B rows ==
  64 uint32 elements — halves the per-row gpsimd charge vs f16; int64
  views would halve it again but the HW ucode mis-executes 8-byte-element
  gathers) and grouped into few large instructions.

Host work is limited to integer index packing / slicing / permutation;
every floating point operation runs on device.
"""
import os
import sys

for _p in ("/opt/trn_rl_repo", "/root/.axon_site/_ro/trn_rl_repo"):
    if os.path.isdir(_p) and _p not in sys.path:
        sys.path.insert(0, _p)

import numpy as np
from contextlib import ExitStack

import concourse.bacc as bacc
import concourse.mybir as mybir
import concourse.tile as tile
from concourse.bass_utils import run_bass_kernel_spmd

P = 128
NCORE = 8
CHUNK = 32768          # dma_gather int16 index reach (rows per chunk)
D = 128                # feature dim (in == out == 128)
GSZ_A = 2              # edge blocks per phase-A gather group
GB_B = 8               # node blocks per phase-B gather group

LAST_EXEC_NS = []
TIME_RUNS = os.environ.get("HGNN_TIME", "0") == "1"

_NC_CACHE = {}
_JIT_CACHE = {}


def _timed_spmd(nc, in_maps, key, reps=32):
    """run_bass_via_pjrt equivalent that stages inputs first and times the
    warm execute (jit cached per nc)."""
    import time as _time
    import jax
    from jax.sharding import Mesh, PartitionSpec, NamedSharding
    from jax.experimental.shard_map import shard_map
    from concourse import bass2jax as b2j
    from concourse import mybir as _mb

    n_cores = len(in_maps)
    partition_name = (nc.partition_id_tensor.name
                     if nc.partition_id_tensor else None)
    in_names, out_names, out_avals, zero_outs = [], [], [], []
    for alloc in nc.m.functions[0].allocations:
        if not isinstance(alloc, _mb.MemoryLocationSet):
            continue
        name = alloc.memorylocations[0].name
        if alloc.kind == "ExternalInput":
            if name != partition_name:
                in_names.append(name)
        elif alloc.kind == "ExternalOutput":
            out_names.append(name)
            shape = tuple(alloc.tensor_shape)
            dtype = _mb.dt.np(alloc.dtype)
            out_avals.append(jax.core.ShapedArray(shape, dtype))
            zero_outs.append(np.zeros(shape, dtype))
    n_params = len(in_names)
    all_in_names = in_names + out_names
    if partition_name is not None:
        all_in_names.append(partition_name)

    def _body(*args):
        operands = list(args)
        if partition_name is not None:
            operands.append(b2j.partition_id_tensor())
        return tuple(b2j._bass_exec_p.bind(
            *operands,
            out_avals=tuple(out_avals),
            in_names=tuple(all_in_names),
            out_names=tuple(out_names),
            lowering_input_output_aliases=(),
            sim_require_finite=True,
            sim_require_nnan=True,
            nc=nc,
        ))

    devices = jax.devices()[:n_cores]
    mesh = Mesh(np.asarray(devices), ("core",))
    sh = NamedSharding(mesh, PartitionSpec("core"))
    staged = [jax.device_put(
        np.concatenate([np.asarray(m[nm]) for m in in_maps], axis=0), sh)
        for nm in in_names]
    staged += [jax.device_put(
        np.zeros((n_cores * z.shape[0], *z.shape[1:]), z.dtype), sh)
        for z in zero_outs]
    jax.block_until_ready(staged)

    if key not in _JIT_CACHE:
        _JIT_CACHE[key] = jax.jit(shard_map(
            _body, mesh=mesh,
            in_specs=(PartitionSpec("core"),) * (n_params + len(out_names)),
            out_specs=(PartitionSpec("core"),) * len(out_names),
            check_rep=False)).lower(*staged).compile()
    fn = _JIT_CACHE[key]

    # Execute repeatedly; report the min — the dispatch path (axon tunnel
    # RPC) adds 30-100 ms of load-dependent jitter on top of the device
    # execution, so a single sample mostly measures tunnel luck.  The first
    # call additionally pays the NEFF device load, which the min discards.
    best_ns = None
    out = None
    for _ in range(reps):
        t0 = _time.perf_counter()
        out = fn(*staged)
        jax.block_until_ready(out)
        dt_ns = int((_time.perf_counter() - t0) * 1e9)
        if best_ns is None or dt_ns < best_ns:
            best_ns = dt_ns
    LAST_EXEC_NS.append(best_ns)

    class _R:
        pass
    r = _R()
    r.results = [
        {nm: np.asarray(out[i]).reshape(n_cores, *out_avals[i].shape)[c]
         for i, nm in enumerate(out_names)}
        for c in range(n_cores)
    ]
    return r


def _wrap16(vals):
    """int16 index wrap: [n] -> [128, n/16] (16-partition wrap, tiled x8)."""
    arr16 = vals.reshape(-1, 16).T.astype(np.int16)
    return np.tile(arr16, (8, 1))


# ----------------------------------------------------------------- packing --
def _pack_edges(node_idx, edge_idx, N, E, d_n, d_e, NCLS):
    """Assign edges to (core, block, slot) bins; group incidences by node
    chunk inside each block.  Returns per-core phase-A arrays + edge slots."""
    n_chunks = (N + CHUNK - 1) // CHUNK
    EBLK = -(-E // (NCORE * P))               # edge blocks per core
    EBLK += (-EBLK) % GSZ_A                   # pad to group multiple
    nbins = NCORE * EBLK

    chunk_of_inc = (node_idx // CHUNK).astype(np.int64)
    prof = np.zeros((E, n_chunks), np.int64)
    np.add.at(prof, (edge_idx, chunk_of_inc), 1)
    tot_per_chunk = prof.sum(axis=0)

    caps = np.maximum(128, ((-(-tot_per_chunk // nbins) + 127) // 128) * 128)

    order = np.argsort(-d_e, kind="stable")
    for _attempt in range(6):
        loads = np.zeros((nbins, n_chunks), np.int64)
        ecnt = np.zeros(nbins, np.int64)
        bin_of_edge = np.full(E, -1, np.int64)
        ok = True
        capsf = caps.astype(np.float64)
        for e in order:
            pe = prof[e]
            cand = loads + pe
            feas = (cand <= caps).all(axis=1) & (ecnt < P)
            if not feas.any():
                ok = False
                break
            score = (cand / capsf).max(axis=1) + ecnt / (P * 4.0)
            score[~feas] = np.inf
            b = int(np.argmin(score))
            bin_of_edge[e] = b
            loads[b] += pe
            ecnt[b] += 1
        if ok:
            break
        caps = caps + 128
    assert ok, "edge packing failed"

    slot_in_bin = np.zeros(E, np.int64)
    cnt = np.zeros(nbins, np.int64)
    for e in order:
        b = bin_of_edge[e]
        slot_in_bin[e] = cnt[b]
        cnt[b] += 1

    eslot = bin_of_edge * P + slot_in_bin      # global ef row per edge

    caps = caps.astype(np.int64)
    T_A = int(caps.sum()) // P                 # tiles per block
    cap_off = np.concatenate([[0], np.cumsum(caps)])

    # --- per-core slot layout -------------------------------------------
    # group-major: [group][chunk][block-in-group][slot in (block,chunk) cap]
    # slot index within core:
    #   base(group) = group * GSZ_A * T_A * P
    #   off(ch, bi) = (cap_off[ch] * GSZ_A + bi * caps[ch]) * 1  (in slots)
    ebin = bin_of_edge[edge_idx]               # [NI]
    key = ebin * n_chunks + chunk_of_inc
    sort = np.argsort(key, kind="stable")
    ks = key[sort]
    grp_start = np.searchsorted(ks, np.arange(nbins * n_chunks), side="left")
    grp_sizes = np.diff(np.concatenate([grp_start, [len(ks)]]))
    assert (grp_sizes.reshape(nbins, n_chunks) <= caps[None, :]).all()
    pos_in_grp = np.arange(len(ks)) - grp_start[ks]

    binid = ks // n_chunks
    chof = ks % n_chunks
    core = binid // EBLK
    blk = binid % EBLK
    grpA = blk // GSZ_A
    bi = blk % GSZ_A

    SLOTS = EBLK * T_A * P                     # incidence slots per core
    flat = (grpA * (GSZ_A * T_A * P)
            + cap_off[chof] * GSZ_A + bi * caps[chof] + pos_in_grp)

    idx_x = np.zeros((NCORE, SLOTS), np.int64)     # chunk-local node row
    idx_oh = np.full((NCORE, SLOTS), NCLS * P, np.int64)  # zero row default
    seg_full = np.full((NCORE, SLOTS), -1.0, np.float32)
    cnt_full = np.ones((NCORE, SLOTS), np.int64)

    inc_sorted = sort
    n_of = node_idx[inc_sorted]
    idx_x[core, flat] = n_of - chof * CHUNK
    idx_oh[core, flat] = (d_n[n_of] - 1) * P + slot_in_bin[edge_idx[inc_sorted]]
    seg_full[core, flat] = slot_in_bin[edge_idx[inc_sorted]]
    cnt_full[core, flat] = d_n[n_of]

    # cntE: [core][P, EBLK] edge sizes per slot (1 for empty slots)
    cntE = np.ones((NCORE, P, EBLK), np.int64)
    cntE[eslot // (EBLK * P), eslot % P, (eslot // P) % EBLK] = np.maximum(d_e, 1)

    idx_x_w = np.stack([_wrap16(idx_x[c]) for c in range(NCORE)])

    # per-(block-in-group, chunk) tile offsets within the group's g tile
    # g tile for a group has GSZ_A * T_A tiles; region (ch, bi) starts at
    # tile (cap_off[ch] * GSZ_A + bi * caps[ch]) / 128, length caps[ch]/128.
    tile_off = np.zeros((GSZ_A, n_chunks), np.int64)
    tile_len = np.zeros(n_chunks, np.int64)
    for ch in range(n_chunks):
        tile_len[ch] = caps[ch] // P
        for b in range(GSZ_A):
            tile_off[b, ch] = (cap_off[ch] * GSZ_A + b * caps[ch]) // P

    # split each block's tiles alternately: even index -> one-hot gathered
    # from the class table (gpsimd), odd -> built on DVE (engine balance)
    TG = GSZ_A * T_A
    G_A = EBLK // GSZ_A
    gcols, dcols = [], []        # group-local gather cols / global DVE cols
    for grp in range(G_A):
        for bi in range(GSZ_A):
            tiles = []
            for ch in range(n_chunks):
                for t in range(int(tile_len[ch])):
                    tiles.append(int(tile_off[bi, ch]) + t)
            for i, t in enumerate(tiles):
                if i % 2 == 0:
                    gcols.append(grp * TG + t)
                else:
                    dcols.append(grp * TG + t)
    gcols = np.asarray(gcols, np.int64)
    dcols = np.asarray(dcols, np.int64)
    NGH = len(gcols) // G_A                    # gather tiles per group
    assert len(gcols) == NGH * G_A
    ND = len(dcols)

    oh_sub = idx_oh.reshape(NCORE, TA_tot := EBLK * T_A, P)[:, gcols, :]         .reshape(NCORE, len(gcols) * P)
    idx_oh_w = np.stack([_wrap16(oh_sub[c]) for c in range(NCORE)])
    segA = seg_full.reshape(NCORE, TA_tot, P)[:, dcols, :]         .transpose(0, 2, 1).copy()             # [NCORE, P, ND] f32
    cntA = cnt_full.reshape(NCORE, TA_tot, P)[:, dcols, :]         .transpose(0, 2, 1).copy()             # [NCORE, P, ND]

    return dict(EBLK=EBLK, T_A=T_A, caps=caps, cap_off=cap_off,
                n_chunks=n_chunks, idx_x=idx_x_w, idx_oh=idx_oh_w,
                segA=segA, cntA=cntA, NGH=NGH, ND=ND,
                cntE=cntE, eslot=eslot, tile_off=tile_off, tile_len=tile_len)


def _pack_nodes(node_idx, edge_idx, N, d_n, eslot, NCLS):
    """Assign nodes to (core, block, slot); incidences grouped by node."""
    order = np.argsort(-d_n, kind="stable")
    core_of_node = np.zeros(N, np.int64)
    snake = np.empty(N, np.int64)
    rounds = -(-N // NCORE)
    pos = 0
    for r in range(rounds):
        blkn = order[r * NCORE:(r + 1) * NCORE]
        if r % 2:
            blkn = blkn[::-1]
        snake[pos:pos + len(blkn)] = blkn
        pos += len(blkn)
    core_seq = np.tile(np.concatenate([np.arange(NCORE), np.arange(NCORE)[::-1]]),
                       rounds // 2 + 1)[:N]
    core_of_node[snake] = core_seq

    TB = 8
    for _ in range(4):
        ok = True
        NBLK = max(1, -(-max(np.bincount(core_of_node, minlength=NCORE).max(), 1)
                        // P))
        while True:
            blk_of_node = np.full(N, -1, np.int64)
            slot_of_node = np.full(N, -1, np.int64)
            ok = True
            for c in range(NCORE):
                nodes = np.where(core_of_node == c)[0]
                deg = d_n[nodes]
                o = np.argsort(-deg, kind="stable")
                nodes = nodes[o]
                deg = deg[o]
                loads = np.zeros(NBLK, np.int64)
                ncnt = np.zeros(NBLK, np.int64)
                bless = np.arange(NBLK)
                for n, dg in zip(nodes, deg):
                    feas = (loads + dg <= TB * P) & (ncnt < P)
                    if not feas.any():
                        ok = False
                        break
                    b = bless[feas][np.argmin(loads[feas])]
                    blk_of_node[n] = b
                    slot_of_node[n] = ncnt[b]
                    loads[b] += dg
                    ncnt[b] += 1
                if not ok:
                    break
            if ok:
                break
            NBLK += 1
            if NBLK > 2 * (-(-N // (NCORE * P))) + 8:
                break
        if ok:
            break
        TB += 1
    assert ok, "node packing failed"
    NBLK += (-NBLK) % GB_B                     # pad to group multiple

    n_of_inc = node_idx
    c_of_inc = core_of_node[n_of_inc]
    b_of_inc = blk_of_node[n_of_inc]
    key = c_of_inc * NBLK + b_of_inc
    sort = np.argsort(key, kind="stable")
    ks = key[sort]
    grp_start = np.searchsorted(ks, np.arange(NCORE * NBLK), side="left")
    pos_in_grp = np.arange(len(ks)) - grp_start[ks]

    SLOTS = NBLK * TB * P
    idx_ef = np.zeros((NCORE, SLOTS), np.int64)
    idx_oh = np.full((NCORE, SLOTS), NCLS * P, np.int64)   # zero row default

    core = ks // NBLK
    blk = ks % NBLK
    flat = blk * (TB * P) + pos_in_grp
    assert (pos_in_grp < TB * P).all()
    idx_ef[core, flat] = eslot[edge_idx[sort]]
    idx_oh[core, flat] = slot_of_node[node_idx[sort]]      # class-1 rows = I

    cntB = np.ones((NCORE, P, NBLK), np.int64)
    valid = blk_of_node >= 0
    cntB[core_of_node[valid], slot_of_node[valid], blk_of_node[valid]] = \
        np.maximum(d_n[valid], 1)

    idx_ef_w = np.stack([_wrap16(idx_ef[c]) for c in range(NCORE)])
    segB = np.where(idx_oh == NCLS * P, -1.0,
                    idx_oh.astype(np.float64)).astype(np.float32)
    segB = segB.reshape(NCORE, NBLK * TB, P).transpose(0, 2, 1).copy()
    # gather-one-hot subset: first TB_G tiles of every block go through the
    # table gather on gpsimd; the rest are built on DVE (engine balance)
    TB_G = max(1, TB // 4)
    oh_sub = idx_oh.reshape(NCORE, NBLK, TB, P)[:, :, :TB_G, :]         .reshape(NCORE, NBLK * TB_G * P)
    idx_oh_w = np.stack([_wrap16(oh_sub[c]) for c in range(NCORE)])

    return dict(NBLK=NBLK, TB=TB, TB_G=TB_G, idx_ef=idx_ef_w, segB=segB,
                idx_oh=idx_oh_w, cntB=cntB,
                core_of_node=core_of_node, blk_of_node=blk_of_node,
                slot_of_node=slot_of_node)


# ------------------------------------------------------------------ kernel --
def _build_neff(N, EBLK, T_A, caps, cap_off, n_chunks, tile_off, tile_len,
                NGH, ND, NBLK, TB, TB_G, NCLS):
    """One NEFF: table build -> phase A -> AllGather(ef) -> phase B."""
    nc = bacc.Bacc("TRN2", target_bir_lowering=False, debug=False,
                   num_devices=NCORE)
    f32, f16 = mybir.dt.float32, mybir.dt.float16
    i16, i32, i64 = mybir.dt.int16, mybir.dt.int32, mybir.dt.int64
    TA_tot = EBLK * T_A
    SLOTS_A = TA_tot * P
    TB_tot = NBLK * TB
    SLOTS_B = TB_tot * P
    NSLOT = NCORE * EBLK * P                   # rows in the full ef table
    G_A = EBLK // GSZ_A
    G_B = NBLK // GB_B
    TROWS = NCLS * P + P                       # table rows, padded block

    xu32 = nc.dram_tensor("xu32", [N, D // 2], mybir.dt.uint32,
                          kind="ExternalInput")
    wt = nc.dram_tensor("wt", [D, D], f16, kind="ExternalInput")      # W.T
    bias = nc.dram_tensor("bias", [1, D], f16, kind="ExternalInput")
    ident = nc.dram_tensor("ident", [P, P], f16, kind="ExternalInput")
    iotaf = nc.dram_tensor("iotaf", [P, P], f16, kind="ExternalInput")
    clsv = nc.dram_tensor("clsv", [P, NCLS], i32, kind="ExternalInput")
    idxax = nc.dram_tensor("idxax", [P, SLOTS_A // 16], i16,
                           kind="ExternalInput")
    idxaoh = nc.dram_tensor("idxaoh", [P, NGH * G_A * P // 16], i16,
                            kind="ExternalInput")
    segA = nc.dram_tensor("segA", [P, ND], f32, kind="ExternalInput")
    cntA = nc.dram_tensor("cntA", [P, ND], i32, kind="ExternalInput")
    cntE = nc.dram_tensor("cntE", [P, EBLK], i32, kind="ExternalInput")
    idxbef = nc.dram_tensor("idxbef", [P, SLOTS_B // 16], i16,
                            kind="ExternalInput")
    segB = nc.dram_tensor("segB", [P, TB_tot], f32, kind="ExternalInput")
    idxboh = nc.dram_tensor("idxboh", [P, NBLK * TB_G * P // 16], i16,
                            kind="ExternalInput")
    cntB = nc.dram_tensor("cntB", [P, NBLK], i32, kind="ExternalInput")
    y = nc.dram_tensor("y", [NBLK * P, D], f32, kind="ExternalOutput")

    tab = nc.dram_tensor("tab", [TROWS, D], f16, kind="Internal")
    ef_loc = nc.dram_tensor("ef_loc", [EBLK * P, D], f16, kind="Internal")
    # collective bounce buffers: input must be Local (collectives cannot
    # read Shared), output Shared for the fast RDH HBM-HBM AllGather path.
    ef_full = nc.dram_tensor("ef_full", [NSLOT, D], f16, kind="Internal",
                             addr_space="Shared")
    tabu = tab[:].bitcast(mybir.dt.uint32)
    efu = ef_full[:].bitcast(mybir.dt.uint32)

    with tile.TileContext(nc) as tc, ExitStack() as ctx:

        const = ctx.enter_context(tc.tile_pool(name="const", bufs=1))
        ident_t = const.tile([P, P], f16)
        nc.sync.dma_start(ident_t[:], ident[:])
        iotaf_t = const.tile([P, P], f16)
        nc.sync.dma_start(iotaf_t[:], iotaf[:])
        wt_t = const.tile([P, D], f16)
        nc.sync.dma_start(wt_t[:], wt[:])
        b_t = const.tile([1, D], f16)
        nc.sync.dma_start(b_t[:], bias[:])
        ones16 = const.tile([P, 1], f16)
        nc.vector.memset(ones16[:], 1.0)

        # ------------- degree-class table: tab[(c-1)*128+j] = rsqrt(c)*e_j
        cls_t = const.tile([P, NCLS], i32)
        nc.sync.dma_start(cls_t[:], clsv[:])
        s_t = const.tile([P, NCLS], f32)
        nc.vector.tensor_copy(s_t[:], cls_t[:])
        nc.vector.reciprocal(s_t[:], s_t[:])
        nc.scalar.sqrt(s_t[:], s_t[:])
        tab_sb = const.tile([P, NCLS + 1, P], f16)
        for c in range(NCLS):
            nc.vector.tensor_scalar(
                out=tab_sb[:, c, :], in0=ident_t[:],
                scalar1=s_t[:, c:c + 1], scalar2=None,
                op0=mybir.AluOpType.mult)
            eng = nc.sync if c % 2 == 0 else nc.scalar
            eng.dma_start(tab[c * P:(c + 1) * P, :], tab_sb[:, c, :])
        nc.vector.memset(tab_sb[:, NCLS, :], 0.0)
        nc.sync.dma_start(tab[NCLS * P:(NCLS + 1) * P, :], tab_sb[:, NCLS, :])

        # ---------------- phase A: nodes -> hyperedges ----------------
        with ExitStack() as actx:
            aconst = actx.enter_context(tc.tile_pool(name="aconst", bufs=1))
            gxp = actx.enter_context(tc.tile_pool(name="agx", bufs=2))
            gohp = actx.enter_context(tc.tile_pool(name="agoh", bufs=2))
            ohpa = actx.enter_context(tc.tile_pool(name="aohp", bufs=12))
            sbp = actx.enter_context(tc.tile_pool(name="asb", bufs=3))
            outp = actx.enter_context(tc.tile_pool(name="aout", bufs=3))
            ppe = actx.enter_context(tc.tile_pool(name="ppe", bufs=2,
                                                  space="PSUM"))
            pq = actx.enter_context(tc.tile_pool(name="pq", bufs=2,
                                                 space="PSUM"))
            ptr = actx.enter_context(tc.tile_pool(name="ptr", bufs=1,
                                                  space="PSUM"))
            pef = actx.enter_context(tc.tile_pool(name="pef", bufs=2,
                                                  space="PSUM"))

            # idx loads on the Act DMA queue: SP is busy with tab writes and
            # these gate the very first gathers.
            idxax_t = aconst.tile([P, SLOTS_A // 16], i16)
            # split: group 0's slice first so the gather stream starts early
            _c0 = GSZ_A * T_A * P // 16
            nc.sync.dma_start(idxax_t[:, 0:_c0], idxax[:, 0:_c0])
            nc.sync.dma_start(idxax_t[:, _c0:], idxax[:, _c0:])
            idxaoh_t = aconst.tile([P, NGH * G_A * P // 16], i16)
            nc.scalar.dma_start(idxaoh_t[:], idxaoh[:])
            sega_t = aconst.tile([P, ND], f32)
            nc.scalar.dma_start(sega_t[:], segA[:])
            cnta_t = aconst.tile([P, ND], i32)
            nc.scalar.dma_start(cnta_t[:], cntA[:])
            sA_t = aconst.tile([P, ND], f32)
            nc.vector.tensor_copy(sA_t[:], cnta_t[:])
            nc.vector.reciprocal(sA_t[:], sA_t[:])
            nc.scalar.sqrt(sA_t[:], sA_t[:])
            cnte_t = aconst.tile([P, EBLK], i32)
            nc.sync.dma_start(cnte_t[:], cntE[:])
            inv_de = aconst.tile([P, EBLK], f32)
            nc.vector.tensor_copy(inv_de[:], cnte_t[:])
            nc.vector.reciprocal(inv_de[:], inv_de[:])

            def a_tail(blk, psum_pe, psum_q):
                """Per-block epilogue: transpose p/q, apply W and bias, scale
                by inv_de, write the ef slice.  Emitted one block late so the
                PE stream never stalls on the psum->SBUF copies."""
                pq_sb = sbp.tile([P, P], f16, tag="pqsb")
                nc.scalar.activation(pq_sb[:], psum_pe[:],
                                     mybir.ActivationFunctionType.Copy)
                q_sb = sbp.tile([P, 1], f16, tag="qsb")
                nc.vector.tensor_copy(q_sb[:], psum_q[:])
                psum_t = ptr.tile([P, P], f32, tag="pt")
                nc.tensor.matmul(psum_t[:], lhsT=pq_sb[:], rhs=ident_t[:],
                                 start=True, stop=True)
                psum_qt = ptr.tile([1, P], f32, tag="pqt")
                nc.tensor.matmul(psum_qt[:], lhsT=q_sb[:], rhs=ident_t[:],
                                 start=True, stop=True)
                pt_sb = sbp.tile([P, P], f16, tag="ptsb")
                nc.scalar.activation(pt_sb[:], psum_t[:],
                                     mybir.ActivationFunctionType.Copy)
                qt_sb = sbp.tile([1, P], f16, tag="qtsb")
                nc.vector.tensor_copy(qt_sb[:], psum_qt[:])
                psum_ef = pef.tile([P, P], f32, tag="ef")
                nc.tensor.matmul(psum_ef[:], lhsT=pt_sb[:], rhs=wt_t[:],
                                 start=True, stop=False)
                nc.tensor.matmul(psum_ef[:], lhsT=qt_sb[:], rhs=b_t[:],
                                 start=False, stop=True)
                out_t = outp.tile([P, P], f16, tag="out")
                nc.scalar.activation(out_t[:], psum_ef[:],
                                     mybir.ActivationFunctionType.Copy,
                                     scale=inv_de[:, blk:blk + 1])
                nc.sync.dma_start(ef_loc[blk * P:(blk + 1) * P, :],
                                  out_t[:])

            TG = GSZ_A * T_A                   # tiles per group
            pending = None
            dcnt = [0]
            for grp in range(G_A):
                base16 = grp * (TG * P) // 16
                gx = gxp.tile([P, TG, D // 2], mybir.dt.uint32, tag="gx")
                for ch in range(n_chunks):
                    cw = int(caps[ch]) * GSZ_A
                    lo = CHUNK * ch
                    hi = min(N, CHUNK * (ch + 1))
                    t0 = int(cap_off[ch]) * GSZ_A // P
                    nc.gpsimd.dma_gather(
                        out_ap=gx[:, t0:t0 + cw // P, :],
                        in_ap=xu32[lo:hi, :],
                        idxs_ap=idxax_t[:, base16 + t0 * P // 16:
                                        base16 + (t0 * P + cw) // 16],
                        num_idxs=cw,
                        num_idxs_reg=cw,
                        elem_size=D // 2,
                        single_packet=False,
                    )
                goh = gohp.tile([P, NGH, D // 2], mybir.dt.uint32,
                                tag="goh")
                nc.gpsimd.dma_gather(
                    out_ap=goh[:],
                    in_ap=tabu,
                    idxs_ap=idxaoh_t[:, grp * NGH * P // 16:
                                     (grp + 1) * NGH * P // 16],
                    num_idxs=NGH * P,
                    num_idxs_reg=NGH * P,
                    elem_size=D // 2,
                    single_packet=False,
                )
                gcnt = 0
                for bi in range(GSZ_A):
                    blk = grp * GSZ_A + bi
                    psum_pe = ppe.tile([P, P], f32, tag="pe")
                    psum_q = pq.tile([P, 1], f32, tag="q")
                    tiles = []
                    for ch in range(n_chunks):
                        for t in range(int(tile_len[ch])):
                            tiles.append(int(tile_off[bi, ch]) + t)
                    ntl = len(tiles)
                    for i, t in enumerate(tiles):
                        if i % 2 == 0:
                            oh16 = goh[:, gcnt, :].bitcast(f16)
                            gcnt += 1
                        else:
                            oh_t = ohpa.tile([P, P], f16, tag="oh")
                            nc.vector.tensor_scalar(
                                out=oh_t[:], in0=iotaf_t[:],
                                scalar1=sega_t[:, dcnt[0]:dcnt[0] + 1],
                                scalar2=sA_t[:, dcnt[0]:dcnt[0] + 1],
                                op0=mybir.AluOpType.is_equal,
                                op1=mybir.AluOpType.mult)
                            dcnt[0] += 1
                            oh16 = oh_t[:]
                        gx16 = gx[:, t, :].bitcast(f16)
                        nc.tensor.matmul(psum_pe[:], lhsT=oh16, rhs=gx16,
                                         start=(i == 0), stop=(i == ntl - 1))
                        nc.tensor.matmul(psum_q[:], lhsT=oh16, rhs=ones16[:],
                                         start=(i == 0), stop=(i == ntl - 1))
                    if pending is not None:
                        a_tail(*pending)
                    pending = (blk, psum_pe, psum_q)
            a_tail(*pending)

        # ---------------- exchange: allgather ef slices ----------------
        nc.gpsimd.collective_compute(
            "AllGather", mybir.AluOpType.bypass,
            replica_groups=[list(range(NCORE))],
            ins=[ef_loc[:].opt()], outs=[ef_full[:].opt()],
        )

        # ---------------- phase B: hyperedges -> nodes ----------------
        with ExitStack() as bctx:
            bconst = bctx.enter_context(tc.tile_pool(name="bconst", bufs=1))
            gefp = bctx.enter_context(tc.tile_pool(name="bgef", bufs=3))
            gohpb = bctx.enter_context(tc.tile_pool(name="bgoh", bufs=3))
            ohpb = bctx.enter_context(tc.tile_pool(name="bohp", bufs=12))
            opoolb = bctx.enter_context(tc.tile_pool(name="bout", bufs=8))
            py = bctx.enter_context(tc.tile_pool(name="py", bufs=8,
                                                 space="PSUM"))

            idxbef_t = bconst.tile([P, SLOTS_B // 16], i16)
            nc.scalar.dma_start(idxbef_t[:], idxbef[:])
            segb_t = bconst.tile([P, TB_tot], f32)
            nc.scalar.dma_start(segb_t[:], segB[:])
            idxboh_t = bconst.tile([P, NBLK * TB_G * P // 16], i16)
            nc.scalar.dma_start(idxboh_t[:], idxboh[:])
            cntb_t = bconst.tile([P, NBLK], i32)
            nc.sync.dma_start(cntb_t[:], cntB[:])
            sb_t = bconst.tile([P, NBLK], f32)
            nc.vector.tensor_copy(sb_t[:], cntb_t[:])
            nc.vector.reciprocal(sb_t[:], sb_t[:])
            nc.scalar.sqrt(sb_t[:], sb_t[:])

            TGB = GB_B * TB
            TGO = GB_B * TB_G
            for grp in range(G_B):
                base16 = grp * (TGB * P) // 16
                obase16 = grp * (TGO * P) // 16
                gef = gefp.tile([P, TGB, D // 2], mybir.dt.uint32,
                                tag="gef")
                nc.gpsimd.dma_gather(
                    out_ap=gef[:],
                    in_ap=efu,
                    idxs_ap=idxbef_t[:, base16:base16 + TGB * P // 16],
                    num_idxs=TGB * P,
                    num_idxs_reg=TGB * P,
                    elem_size=D // 2,
                    single_packet=False,
                )
                goh = gohpb.tile([P, TGO, D // 2], mybir.dt.uint32,
                                 tag="goh")
                nc.gpsimd.dma_gather(
                    out_ap=goh[:],
                    in_ap=tabu,
                    idxs_ap=idxboh_t[:, obase16:obase16 + TGO * P // 16],
                    num_idxs=TGO * P,
                    num_idxs_reg=TGO * P,
                    elem_size=D // 2,
                    single_packet=False,
                )
                for bi in range(GB_B):
                    blk = grp * GB_B + bi
                    psum_y = py.tile([P, P], f32, tag="py")
                    for t in range(TB):
                        tt = bi * TB + t
                        if t < TB_G:
                            oh16 = goh[:, bi * TB_G + t, :].bitcast(f16)
                        else:
                            oh_t = ohpb.tile([P, P], f16, tag="oh")
                            nc.vector.tensor_scalar(
                                out=oh_t[:], in0=iotaf_t[:],
                                scalar1=segb_t[:, grp * TGB + tt:
                                               grp * TGB + tt + 1],
                                scalar2=None, op0=mybir.AluOpType.is_equal)
                            oh16 = oh_t[:]
                        ge16 = gef[:, tt, :].bitcast(f16)
                        nc.tensor.matmul(psum_y[:], lhsT=oh16, rhs=ge16,
                                         start=(t == 0), stop=(t == TB - 1))
                    out_t = opoolb.tile([P, P], f32, tag="out")
                    nc.scalar.activation(out_t[:], psum_y[:],
                                         mybir.ActivationFunctionType.Relu,
                                         scale=sb_t[:, blk:blk + 1])
                    nc.sync.dma_start(y[blk * P:(blk + 1) * P, :], out_t[:])
    nc.compile()
    return nc


# -------------------------------------------------------------------- main --
def _prepare(x, W, b, node_idx, edge_idx, N, E):
    """Pack the graph, build/compile the NEFF, stage per-core inputs."""
    d_n = np.bincount(node_idx, minlength=N)
    d_e = np.bincount(edge_idx, minlength=E)
    NCLS = int(max(d_n.max(), 1))
    assert NCLS * P + P <= 32768, "degree classes exceed int16 gather reach"

    pa = _pack_edges(node_idx, edge_idx, N, E, d_n, d_e, NCLS)
    pb = _pack_nodes(node_idx, edge_idx, N, d_n, pa["eslot"], NCLS)

    EBLK, T_A = pa["EBLK"], pa["T_A"]
    NBLK, TB, TB_G = pb["NBLK"], pb["TB"], pb["TB_G"]

    key = ("GOH", N, EBLK, T_A, tuple(pa["caps"].tolist()), NBLK, TB, TB_G,
           NCLS)
    if key not in _NC_CACHE:
        _NC_CACHE[key] = _build_neff(
            N, EBLK, T_A, pa["caps"], pa["cap_off"], pa["n_chunks"],
            pa["tile_off"], pa["tile_len"], pa["NGH"], pa["ND"],
            NBLK, TB, TB_G, NCLS)

    xu32 = np.ascontiguousarray(x.astype(np.float16)).view(np.uint32)
    wt16 = np.ascontiguousarray(W.T).astype(np.float16)
    b16 = b.astype(np.float16)
    ident = np.eye(P, dtype=np.float16)
    iotaf = np.tile(np.arange(P, dtype=np.float16), (P, 1))
    clsv = np.tile(np.arange(1, NCLS + 1, dtype=np.int32), (P, 1))

    in_maps = []
    for c in range(NCORE):
        in_maps.append({
            "xu32": xu32, "wt": wt16, "bias": b16, "ident": ident,
            "iotaf": iotaf, "clsv": clsv,
            "idxax": pa["idx_x"][c], "idxaoh": pa["idx_oh"][c],
            "segA": pa["segA"][c], "cntA": pa["cntA"][c].astype(np.int32),
            "cntE": pa["cntE"][c].astype(np.int32),
            "idxbef": pb["idx_ef"][c], "segB": pb["segB"][c],
            "idxboh": pb["idx_oh"][c],
            "cntB": pb["cntB"][c].astype(np.int32),
        })
    return _NC_CACHE[key], key, in_maps, pb


def kernel(x, W, b, node_idx, edge_idx, num_nodes=None, num_edges=None,
           **_ignored):
    x = np.asarray(x, np.float32)
    W = np.asarray(W, np.float32)
    b = np.asarray(b, np.float32).reshape(1, -1)
    node_idx = np.asarray(node_idx).astype(np.int64).ravel()
    edge_idx = np.asarray(edge_idx).astype(np.int64).ravel()
    N = int(num_nodes) if num_nodes is not None else x.shape[0]
    E = int(num_edges) if num_edges is not None else int(edge_idx.max()) + 1

    ncAB, key, in_maps, pb = _prepare(x, W, b, node_idx, edge_idx, N, E)

    if TIME_RUNS:
        res = _timed_spmd(ncAB, in_maps, key)
    else:
        res = run_bass_kernel_spmd(ncAB, in_maps,
                                   core_ids=list(range(NCORE)))

    y_dev = np.stack([res.results[c]["y"] for c in range(NCORE)])
    out = y_dev[pb["core_of_node"],
                pb["blk_of_node"] * P + pb["slot_of_node"], :]
    return np.ascontiguousarray(out, dtype=np.float32)
